# revision 5
# baseline (speedup 1.0000x reference)
"""Trainium2 Bass kernel: fused store_kvcache + causal prefill attention.

Problem (hardcoded): T=8192 tokens, H=16 heads, D=128, seq_len=2048 (B=4
packed sequences), fp32 in/out. slot_mapping is arange(T) (contiguous slots),
so the KV-cache scatter followed by the cache gather is an identity
permutation on [0,T): attention reads exactly k/v. For robustness, any
non-identity slot_mapping is materialized on the host before the device call.

Sharding: tensor-parallel over heads. 16 heads / 8 NeuronCores = 2 heads per
core; each core runs the same Bass program on its own head slice (SPMD).
Host-side prep per core: slice the 2 heads and lay Q/K out d-major
([head, batch, d, token]) in bf16 — the layout the PE contraction needs.

Per (batch, head) the device computes, flash-attention style per 512-query
block (bf16 matmul operands, fp32 PSUM accumulation):
  S^T[kj,qi] = (K^T_j)^T @ Q^T          (PE, N=512 moving)
  P^T        = exp(SCALE * S^T)         (ACT, PSUM->SBUF bf16; causal mask
                                         applied on diagonal tiles via DVE)
  acc       += P^T_j                    (DVE bf16, softmax denominator prep)
  O^T       += V_j-stationary matmul    (PE, accumulate over kj tiles)
  rowsum_c   = acc_chunk^T @ ones       (PE, N=1, per 128-query chunk)
  O          = transpose(O^T) * (1/rowsum)  (PE transpose + DVE scale)
"""

import numpy as np
import ml_dtypes

import concourse.bass as bass
import concourse.bacc as bacc
import concourse.tile as tile
from concourse import mybir
from concourse.bass_utils import run_bass_kernel_spmd
from concourse.masks import make_identity

# Problem constants (match the grading harness inputs).
T, H, D = 8192, 16, 128
SEQ_LEN = 2048
NUM_SLOTS = 16384
SCALE = 0.08838834764831845  # 1/sqrt(128)
N_CORES = 8
HPC = H // N_CORES  # heads per core
B = T // SEQ_LEN

BF16 = mybir.dt.bfloat16
F32 = mybir.dt.float32

QBLK = 512           # query block (one PSUM bank of fp32)
NMI = QBLK // 128    # 128-chunks per query block


def build_attention(nc, qT_d, kT_d, vh, masks, oh, S, B_, HPC_):
    """Emit the Tile program.

    qT_d/kT_d: DRAM APs [HPC_, B_, 128, S] bf16 (d-major Q/K).
    vh:        DRAM AP [B_*S, HPC_, 128] fp32 (natural V).
    masks:     DRAM AP [128, NMI, QBLK] bf16 (diagonal-tile causal masks).
    oh:        DRAM AP [B_*S, HPC_, 128] fp32 output.
    """
    NT = S // 128           # 128-token tiles per sequence
    NBLK = S // QBLK        # query blocks per sequence

    with tile.TileContext(nc) as tc:
        with (
            tc.tile_pool(name="singles", bufs=1) as singles,
            tc.tile_pool(name="dmaj", bufs=2) as dmaj,
            tc.tile_pool(name="ptp", bufs=4) as ptp,
            tc.tile_pool(name="accp", bufs=2) as accp,
            tc.tile_pool(name="outp", bufs=2) as outp,
            tc.tile_pool(name="ps_s", bufs=3, space="PSUM") as ps_s,
            tc.tile_pool(name="ps_o", bufs=2, space="PSUM") as ps_o,
            tc.tile_pool(name="ps_r", bufs=1, space="PSUM") as ps_r,
            tc.tile_pool(name="ps_t", bufs=2, space="PSUM") as ps_t,
        ):
            masks_sb = singles.tile([128, NMI, QBLK], BF16)
            nc.sync.dma_start(out=masks_sb, in_=masks)
            ident = singles.tile([128, 128], F32)
            make_identity(nc, ident)
            ones = singles.tile([128, 1], BF16)
            nc.vector.memset(ones, 1.0)

            for b in range(B_):
                for h in range(HPC_):
                    base = b * S
                    # d-major Q/K: straight HWDGE loads, contiguous 4KB rows
                    qT = dmaj.tile([128, NT, 128], BF16, tag="qT")
                    nc.sync.dma_start(
                        out=qT, in_=qT_d[h, b].rearrange("d (n p) -> d n p", p=128)
                    )
                    kT = dmaj.tile([128, NT, 128], BF16, tag="kT")
                    nc.sync.dma_start(
                        out=kT, in_=kT_d[h, b].rearrange("d (n p) -> d n p", p=128)
                    )
                    # natural V tiles, fp32->bf16 cast in the SWDGE datapath
                    vsrc = vh[base : base + S, h, :].rearrange(
                        "(n p) d -> p n d", p=128
                    )
                    vsb = dmaj.tile([128, NT, 128], BF16, tag="vsb")
                    nc.gpsimd.dma_start(out=vsb, in_=vsrc)

                    for blk in range(NBLK):
                        nj = (blk + 1) * NMI
                        o_ps = ps_o.tile([128, QBLK], F32, tag="o_ps")
                        acc = accp.tile([128, QBLK], BF16, tag="acc")
                        qmov = qT[:, blk * NMI : (blk + 1) * NMI, :]

                        # software-pipelined: QK(j) runs ahead of the
                        # exp/mask/accumulate/PV tail of j-1 on the PE stream
                        s_tiles = [None] * nj

                        def emit_qk(j):
                            s_ps = ps_s.tile([128, QBLK], F32, tag="s_ps")
                            nc.tensor.matmul(
                                s_ps, lhsT=kT[:, j, :], rhs=qmov,
                                start=True, stop=True,
                            )
                            s_tiles[j] = s_ps

                        def emit_tail(j):
                            pT = ptp.tile([128, QBLK], BF16, tag="pT")
                            nc.scalar.activation(
                                out=pT, in_=s_tiles[j],
                                func=mybir.ActivationFunctionType.Exp,
                                scale=SCALE,
                            )
                            if j >= blk * NMI:
                                mi = j - blk * NMI
                                nc.vector.tensor_mul(
                                    pT, pT, masks_sb[:, mi, :]
                                )
                            if j == 0:
                                nc.vector.tensor_copy(acc, pT)
                            else:
                                nc.vector.tensor_add(acc, acc, pT)
                            nc.tensor.matmul(
                                o_ps, lhsT=vsb[:, j, :], rhs=pT,
                                start=(j == 0), stop=(j == nj - 1),
                                skip_group_check=True,
                            )

                        emit_qk(0)
                        for j in range(1, nj):
                            emit_qk(j)
                            emit_tail(j - 1)
                        emit_tail(nj - 1)

                        # ---- softmax denominators: rowsum per 128-chunk ----
                        r_ps = ps_r.tile([128, NMI], F32, tag="r_ps")
                        for c in range(NMI):
                            nc.tensor.matmul(
                                r_ps[:, c : c + 1],
                                lhsT=acc[:, c * 128 : (c + 1) * 128],
                                rhs=ones, start=True, stop=True,
                                skip_group_check=True,
                            )
                        recip = outp.tile([128, NMI], F32, tag="recip")
                        nc.vector.reciprocal(recip, r_ps)

                        # ---- O^T -> O, normalize, store ----
                        oT_sb = outp.tile([128, QBLK], F32, tag="oT_sb")
                        nc.vector.tensor_copy(oT_sb, o_ps)
                        o_sb = outp.tile([128, NMI, 128], F32, tag="o_sb")
                        for c in range(NMI):
                            t_ps = ps_t.tile([128, 128], F32, tag="t_ps")
                            nc.tensor.transpose(
                                t_ps, oT_sb[:, c * 128 : (c + 1) * 128], ident
                            )
                            nc.vector.tensor_scalar_mul(
                                o_sb[:, c, :], t_ps, recip[:, c : c + 1]
                            )
                        r0 = base + blk * QBLK
                        odst = oh[r0 : r0 + QBLK, h, :].rearrange(
                            "(c p) d -> p c d", p=128
                        )
                        nc.gpsimd.dma_start(out=odst, in_=o_sb)


def build_masks(S=SEQ_LEN):
    """Diagonal-tile causal masks: masks[p, mi, y] = 1 if y >= p + 128*mi."""
    p = np.arange(128)[:, None]
    y = np.arange(QBLK)[None, :]
    m = np.stack([(y >= p + 128 * mi) for mi in range(NMI)], axis=1)
    return m.astype(ml_dtypes.bfloat16)


_CACHED = {}


def _get_program():
    if "nc" not in _CACHED:
        nc = bacc.Bacc("TRN2", target_bir_lowering=False)
        qT_d = nc.dram_tensor(
            "qTh", [HPC, B, D, SEQ_LEN], BF16, kind="ExternalInput"
        ).ap()
        kT_d = nc.dram_tensor(
            "kTh", [HPC, B, D, SEQ_LEN], BF16, kind="ExternalInput"
        ).ap()
        vh = nc.dram_tensor("vh", [T, HPC, D], F32, kind="ExternalInput").ap()
        masks = nc.dram_tensor(
            "masks", [128, NMI, QBLK], BF16, kind="ExternalInput"
        ).ap()
        oh = nc.dram_tensor("oh", [T, HPC, D], F32, kind="ExternalOutput").ap()
        build_attention(nc, qT_d, kT_d, vh, masks, oh, SEQ_LEN, B, HPC)
        nc.compile()  # bacc passes: split >1-wait syncs into event semaphores
        _CACHED["nc"] = nc
    return _CACHED["nc"]


def _host_resolve_kv(k, v, k_cache, v_cache, slot_mapping):
    """Apply the cache scatter+gather on the host iff it is not the identity."""
    sm = np.asarray(slot_mapping)
    if sm.shape == (T,) and np.array_equal(sm, np.arange(T, dtype=sm.dtype)):
        return k, v
    kc = np.array(k_cache, dtype=np.float32, copy=True)
    vc = np.array(v_cache, dtype=np.float32, copy=True)
    valid = sm >= 0
    kc[sm[valid]] = k.reshape(T, H * D)[valid]
    vc[sm[valid]] = v.reshape(T, H * D)[valid]
    return kc[:T].reshape(T, H, D), vc[:T].reshape(T, H, D)


def _dmajor(x):
    """[T, H, D] fp32 -> [H, B, D, S] bf16 (d-major per sequence)."""
    xb = x.astype(ml_dtypes.bfloat16)
    return np.ascontiguousarray(
        xb.reshape(B, SEQ_LEN, H, D).transpose(2, 0, 3, 1)
    )


def kernel(q, k, v, k_cache, v_cache, slot_mapping, seq_len, _trace=False,
           _trace_kwargs=None):
    q = np.asarray(q, dtype=np.float32)
    k = np.asarray(k, dtype=np.float32)
    v = np.asarray(v, dtype=np.float32)
    assert q.shape == (T, H, D), q.shape
    assert int(seq_len) == SEQ_LEN, seq_len

    k, v = _host_resolve_kv(k, v, np.asarray(k_cache), np.asarray(v_cache),
                            slot_mapping)

    qTm = _dmajor(q)  # [H, B, D, S] bf16
    kTm = _dmajor(k)
    masks = build_masks()
    nc = _get_program()
    in_maps = []
    for c in range(N_CORES):
        hs = slice(c * HPC, (c + 1) * HPC)
        in_maps.append({
            "qTh": np.ascontiguousarray(qTm[hs]),
            "kTh": np.ascontiguousarray(kTm[hs]),
            "vh": np.ascontiguousarray(v[:, hs, :]),
            "masks": masks,
        })
    res = run_bass_kernel_spmd(
        nc, in_maps, core_ids=list(range(N_CORES)),
        trace=_trace, **(_trace_kwargs or {}),
    )
    out = np.empty((T, H, D), dtype=np.float32)
    for c in range(N_CORES):
        out[:, c * HPC : (c + 1) * HPC, :] = res.results[c]["oh"]
    if _trace:
        kernel.last_results = res
    return out
